# revision 2
# baseline (speedup 1.0000x reference)
"""Trainium2 Bass kernel for hierarchical graph-triple attention.

Reference computation (per batch b):
    inter = h @ W.T                                   [B,E]
    beta[g,t] = sum_e emb[g,t,e] * inter[e]           (emb masked)
    p = softmax over t of (beta where valid else -inf); p=0 at invalid
    alpha = alpha_top[:,None] * p                     [B,G,T]  (output)
    c = sum_{g,t} alpha[g,t] * emb[g,t,:]             [B,E]    (output)

Key algebraic fact: the mask never needs to touch the big emb tensor.
Masking beta before the softmax makes alpha exactly 0 at invalid
triples, so c = sum alpha*raw_emb is exact.

Sharding: data-parallel over B (4 batches per core, 8 cores).
Per core, batches are processed as 2 pairs; each pair's emb slab is
streamed as 8 x [128, 2048] fp32 chunks (row = (batch,graph), free =
(t,e)).  beta is computed by fused DVE tensor_tensor_reduce dots,
softmax runs along the free dim, and c accumulates on TensorE via
per-t matmuls with a zero-padded [128,2] stationary operand that
yields both batches of the pair in one accumulation chain.
"""

import sys

sys.path.insert(0, "/opt/trn_rl_repo")

import numpy as np

B, G, T, E, H = 32, 64, 64, 256, 1024
NCORES = 8
BPC = B // NCORES          # batches per core
PAIRS = BPC // 2           # batch-pairs per core
CHUNK_T = 8                # t-columns per DMA chunk
CHUNK_F = CHUNK_T * E      # 2048 floats per chunk row
NCHUNK = T // CHUNK_T      # chunks per pair
NEGBIG = 300.0             # additive mask; exp(beta-300) underflows to 0

_PROGRAM = None


def _build_program():
    import concourse.bacc as bacc
    import concourse.mybir as mybir
    from concourse.bass import MemorySpace
    from concourse.tile import TileContext

    f32 = mybir.dt.float32
    i32 = mybir.dt.int32
    ADD = mybir.AluOpType.add
    MULT = mybir.AluOpType.mult

    nc = bacc.Bacc("TRN2", num_devices=NCORES, debug=False, enable_asserts=False)

    emb = nc.dram_tensor("emb", [BPC, G, T, E], f32, kind="ExternalInput").ap()
    wt = nc.dram_tensor("wt", [128, 8 * E], f32, kind="ExternalInput").ap()
    hrep = nc.dram_tensor("hrep", [128, PAIRS * 8 * 128], f32, kind="ExternalInput").ap()
    maskin = nc.dram_tensor("mask", [BPC, G, T], i32, kind="ExternalInput").ap()
    atop = nc.dram_tensor("atop", [128, PAIRS], f32, kind="ExternalInput").ap()
    c_out = nc.dram_tensor("c_out", [BPC, E], f32, kind="ExternalOutput").ap()
    alpha_out = nc.dram_tensor("alpha_out", [BPC, G, T], f32, kind="ExternalOutput").ap()

    emb_flat = emb.rearrange("b g t e -> (b g) (t e)")      # [BPC*G, T*E]
    mask_flat = maskin.rearrange("b g t -> (b g) t")        # [BPC*G, T]
    alpha_flat = alpha_out.rearrange("b g t -> (b g) t")    # [BPC*G, T]

    with TileContext(nc) as tc:
        with (
            tc.tile_pool(name="const", bufs=1) as constp,
            tc.tile_pool(name="chunks", bufs=2 * NCHUNK) as chunkp,
            tc.tile_pool(name="scratch", bufs=2) as scratchp,
            tc.tile_pool(name="small", bufs=2) as smallp,
            tc.tile_pool(name="psum_i", bufs=2, space=MemorySpace.PSUM) as psumip,
            tc.tile_pool(name="psum_c", bufs=2, space=MemorySpace.PSUM) as psumcp,
        ):
            wt_sb = constp.tile([128, 8 * E], f32)
            nc.sync.dma_start(wt_sb[:], wt[:])
            hrep_sb = constp.tile([128, PAIRS * 8 * 128], f32)
            nc.sync.dma_start(hrep_sb[:], hrep[:])
            atop_sb = constp.tile([128, PAIRS], f32)
            nc.sync.dma_start(atop_sb[:], atop[:])

            for p in range(PAIRS):
                rows = slice(p * 128, (p + 1) * 128)

                # inter[b,e] replicated to [128,E]: rows 0-63 = batch 2p,
                # rows 64-127 = batch 2p+1 (hrep columns pre-replicated on host)
                psum_i = psumip.tile([128, E], f32)
                for c in range(8):
                    nc.tensor.matmul(
                        psum_i[:],
                        hrep_sb[:, (p * 8 + c) * 128 : (p * 8 + c + 1) * 128],
                        wt_sb[:, c * E : (c + 1) * E],
                        start=(c == 0),
                        stop=(c == 7),
                    )
                inter = smallp.tile([128, E], f32, tag="inter")
                nc.scalar.copy(inter[:], psum_i[:])

                # additive mask: 0 where valid, -NEGBIG where invalid
                mask_f = smallp.tile([128, T], f32, tag="maskf")
                nc.gpsimd.dma_start(mask_f[:], mask_flat[rows, :])  # i32->f32 cast
                negm = smallp.tile([128, T], f32, tag="negm")
                nc.vector.tensor_scalar(
                    negm[:], mask_f[:], -1.0, NEGBIG, op0=ADD, op1=MULT
                )

                beta = smallp.tile([128, T], f32, tag="beta")
                chunks = []
                for j in range(NCHUNK):
                    ch = chunkp.tile([128, CHUNK_F], f32, tag="ch")
                    nc.sync.dma_start(
                        ch[:], emb_flat[rows, j * CHUNK_F : (j + 1) * CHUNK_F]
                    )
                    chunks.append(ch)
                    for s in range(CHUNK_T):
                        t_idx = j * CHUNK_T + s
                        sc = scratchp.tile([128, E], f32, tag="ttr")
                        nc.vector.scalar_tensor_tensor(
                            out=sc[:],
                            in0=ch[:, s * E : (s + 1) * E],
                            scalar=1.0,
                            in1=inter[:],
                            op0=MULT,
                            op1=MULT,
                            accum_out=beta[:, t_idx : t_idx + 1],
                        )

                # masked softmax along t (no max-subtraction: |beta| <~ 50)
                badd = smallp.tile([128, T], f32, tag="badd")
                nc.vector.tensor_add(badd[:], beta[:], negm[:])
                ex = smallp.tile([128, T], f32, tag="ex")
                nc.scalar.activation(ex[:], badd[:], mybir.ActivationFunctionType.Exp)
                exm = smallp.tile([128, T], f32, tag="exm")
                nc.vector.tensor_mul(exm[:], ex[:], mask_f[:])
                sm = smallp.tile([128, 1], f32, tag="sm")
                nc.vector.reduce_sum(sm[:], exm[:], axis=mybir.AxisListType.X)
                rec = smallp.tile([128, 1], f32, tag="rec")
                nc.vector.reciprocal(rec[:], sm[:])
                srow = smallp.tile([128, 1], f32, tag="srow")
                nc.vector.tensor_mul(srow[:], rec[:], atop_sb[:, p : p + 1])

                # w = alpha_top * p, zero-padded into [128, 2, T] so one
                # K=128 matmul per t yields both batches' c rows
                wpad = smallp.tile([128, 2, T], f32, tag="wpad")
                nc.gpsimd.memset(wpad[:], 0.0)
                nc.vector.tensor_scalar_mul(wpad[0:64, 0, :], exm[0:64, :], srow[0:64, :])
                nc.vector.tensor_scalar_mul(
                    wpad[64:128, 1, :], exm[64:128, :], srow[64:128, :]
                )

                nc.scalar.dma_start(alpha_flat[p * 128 : p * 128 + 64, :], wpad[0:64, 0, :])
                nc.scalar.dma_start(
                    alpha_flat[p * 128 + 64 : (p + 1) * 128, :], wpad[64:128, 1, :]
                )

                psum_c = psumcp.tile([2, E], f32)
                for t_idx in range(T):
                    j, s = divmod(t_idx, CHUNK_T)
                    nc.tensor.matmul(
                        psum_c[:],
                        wpad[:, :, t_idx],
                        chunks[j][:, s * E : (s + 1) * E],
                        start=(t_idx == 0),
                        stop=(t_idx == T - 1),
                    )
                csb = smallp.tile([2, E], f32, tag="csb")
                nc.scalar.copy(csb[:], psum_c[:])
                nc.scalar.dma_start(c_out[2 * p : 2 * p + 2, :], csb[:])

    nc.compile()
    return nc


def _get_program():
    global _PROGRAM
    if _PROGRAM is None:
        _PROGRAM = _build_program()
    return _PROGRAM


def _make_in_maps(decoder_hidden_state, alpha_graph_attention_top, all_embeddings, W, triple_mask):
    h = np.asarray(decoder_hidden_state, dtype=np.float32)
    atop_full = np.asarray(alpha_graph_attention_top, dtype=np.float32)
    emb_full = np.asarray(all_embeddings, dtype=np.float32)
    W_np = np.asarray(W, dtype=np.float32)
    mask_full = np.asarray(triple_mask, dtype=np.int32)

    # wt[p, c*E+e] = W.T[c*128+p, e]  (stationary operand layout for inter)
    wt_host = np.ascontiguousarray(
        W_np.T.reshape(8, 128, E).transpose(1, 0, 2).reshape(128, 8 * E)
    )

    in_maps = []
    for core in range(NCORES):
        b0 = core * BPC
        h_core = h[b0 : b0 + BPC]                      # [BPC, H]
        # hrep[p, ((pair*8)+c)*128 + m] = h_core[2*pair + m//64, c*128+p]
        h4 = h_core.reshape(PAIRS, 2, 8, 128)          # [pair, b, c, p]
        hr = np.transpose(h4, (3, 0, 2, 1))            # [p, pair, c, b]
        hr = np.repeat(hr, 64, axis=3)                 # [p, pair, c, 128]
        hrep_core = np.ascontiguousarray(hr.reshape(128, PAIRS * 8 * 128))
        # atop[:, pair] = concat(atop[2*pair], atop[2*pair+1]) over g
        atop_core = np.ascontiguousarray(
            atop_full[b0 : b0 + BPC].reshape(PAIRS, 128).T
        )
        in_maps.append(
            {
                "emb": np.ascontiguousarray(emb_full[b0 : b0 + BPC]),
                "wt": wt_host,
                "hrep": hrep_core,
                "mask": np.ascontiguousarray(mask_full[b0 : b0 + BPC]),
                "atop": atop_core,
            }
        )
    return in_maps


def run(inputs: dict, trace: bool = False, trace_kwargs: dict | None = None):
    """Compile (cached), run on 8 cores, return (results_object, (c, alpha))."""
    from concourse.bass_utils import run_bass_kernel_spmd

    nc = _get_program()
    in_maps = _make_in_maps(**inputs)
    res = run_bass_kernel_spmd(
        nc,
        in_maps,
        list(range(NCORES)),
        trace=trace,
        **(trace_kwargs or {}),
    )
    c = np.concatenate([r["c_out"] for r in res.results], axis=0)
    alpha = np.concatenate([r["alpha_out"] for r in res.results], axis=0)
    return res, (c, alpha)


def kernel(**inputs):
    _, out = run(inputs, trace=False)
    return out
